# revision 18
# baseline (speedup 1.0000x reference)
"""Fused multi-head attention on 8 TRN2 NeuronCores.

Problem: x[2,2048,1024] -> q,k,v = x@W.T+b (16 heads x 64), softmax(q k^T/8) v,
then out @ Wp.T + bp.

Sharding: data-parallel over batch (2) x tensor-parallel over heads (4 ranks x
4 heads = 256 dims, Megatron-style).  Core c handles batch c//4, head-rank c%4.
The proj partial sums are reduced on the host (numpy), and the v-bias and
proj-bias are folded into one host-side vector bp_eff = bv @ Wp.T + bp.

Per-core layouts (host pre-transposes/pre-tiles, all DMA rows are >=512B
contiguous DRAM runs):
  xT  [1024, 2048]  x[b].T
  wqT/wkT/wvT [128, 8*256]  W.T slice pre-tiled so partition p holds all 8
                            contraction tiles contiguously
  wpT [256, 1024]           Wp.T rows for this rank's 256 dims
  bq/bk [256, 1]
  outT [1024, 2048] bf16 partial (x[b] @ ..).T, missing bv/bp contributions

Kernel math per core (all matmul operands bfloat16, fp32 PSUM accumulate):
  qT = wqT.T @ xT + bq   [256, 2048]  (transposed layout, d on partitions)
  kT = wkT.T @ xT + bk   [256, 2048]
  v  = xT.T @ wvT        [2048, 256]  packed per key-block with a block of
                         ones appended: vpk[nt] = [128 keys, 4 heads, 128]
                         where cols 0:64 = v, cols 64:128 = 1.0
  attention runs as ONE flat software-pipelined stream over all 128
  (n-chunk, head-pair, key-block) blocks; per block:
     sT[m, n] = kT.T @ qT    two heads row-packed (K=64 matmuls)
     p = exp(sT / 8)         ACT, one [128,1024] instr, both heads
     po_h[:, n] += vpk[mb,h].T @ p_h   per head: partitions 0:64 = P@V,
                              partitions 64:128 = softmax denominator
                              (the ones-columns), i.e. the denominator
                              rides along in the otherwise-idle half of
                              the PE output -- no separate den matmuls.
  PV trails its block's exp by one position globally, so the in-order
  PE always has the next score matmuls queued while ACT runs exp.  The
  q/k/v projections and the output projection are emitted as "filler"
  generators pumped one slice per block inside the stream, absorbing the
  PE idle gaps of the ACT-paced phase.
  attnT = po[0:64] * reciprocal_approx_fast(po[64:128]) per head
  outT += wpT.T @ attnT    [1024, n-chunk], per 128-row tile, bf16 out
"""

import numpy as np

DIM = 1024
N_TOK = 2048
N_HEADS_LOC = 4       # heads per core
D_LOC = 256           # local q/k/v dims per core
SCALE = 64 ** -0.5
P = 128
CH = 512              # n-chunk (moving free dim)
NCH = N_TOK // CH     # 4
KT = DIM // P         # 8 contraction tiles for qkv/proj
MB = N_TOK // P       # 16 key blocks
N_CORES = 8

_NC_CACHE = {}


def build_nc(dt_mm_name="bfloat16"):
    import concourse.mybir as mybir
    import concourse.tile as tile
    from concourse import bacc
    from concourse.bass import ts

    f32 = mybir.dt.float32
    dt_mm = getattr(mybir.dt, dt_mm_name)
    Exp = mybir.ActivationFunctionType.Exp

    nc = bacc.Bacc("TRN2", target_bir_lowering=False, debug=False,
                   num_devices=N_CORES)
    xT = nc.dram_tensor("xT", [DIM, N_TOK], dt_mm, kind="ExternalInput").ap()
    wqT = nc.dram_tensor("wqT", [P, KT * D_LOC], dt_mm, kind="ExternalInput").ap()
    wkT = nc.dram_tensor("wkT", [P, KT * D_LOC], dt_mm, kind="ExternalInput").ap()
    wvT = nc.dram_tensor("wvT", [P, KT * D_LOC], dt_mm, kind="ExternalInput").ap()
    wpT = nc.dram_tensor("wpT", [D_LOC, DIM], dt_mm, kind="ExternalInput").ap()
    bq = nc.dram_tensor("bq", [D_LOC, 1], f32, kind="ExternalInput").ap()
    bk = nc.dram_tensor("bk", [D_LOC, 1], f32, kind="ExternalInput").ap()
    outT = nc.dram_tensor("outT", [DIM, N_TOK], dt_mm, kind="ExternalOutput").ap()

    with tile.TileContext(nc) as tc:
        with (
            tc.tile_pool(name="const", bufs=1) as const,
            tc.tile_pool(name="work", bufs=2) as work,
            # scores + pre-stream k00/q00 accumulators: 2x [128,1024] = 4 banks
            tc.tile_pool(name="ps_s", bufs=2, space="PSUM") as ps_s,
            # filler ring (proj halves, vproj, outproj): 2x [128,512] = 2 banks
            tc.tile_pool(name="ps_pj", bufs=2, space="PSUM") as ps_pj,
            # PV+den accumulators, one per head of the active pair: 2 banks
            tc.tile_pool(name="ps_po", bufs=1, space="PSUM") as ps_po,
        ):
            # ---- persistent SBUF state ----
            w_tiles = {}
            for name in ("k", "q", "v"):
                w_tiles[name] = const.tile([P, KT, D_LOC], dt_mm, tag=f"w{name}",
                                           name=f"w{name}")
            x_sb = [const.tile([P, N_TOK], dt_mm, tag=f"x{i}", name=f"x{i}")
                    for i in range(KT)]

            # DMA issue order: each engine's dma_starts serialize (~0.65us
            # each) and the 16 queues round-robin all in-flight transfers, so
            # total wire time (~6MB at ~320GB/s aggregate = 19us) bounds the
            # head.  Only x/wk/wq (+biases) gate the pre-stream, so they go
            # first; wv is needed one block into the stream and wp only at
            # position 3, so they queue behind everything else.
            def w_slice(dst, src_ap, k0, k1):
                nc.sync.dma_start(
                    out=dst[:, k0:k1, :],
                    in_=src_ap[:, k0 * D_LOC:k1 * D_LOC].rearrange(
                        "p (k n) -> p k n", k=k1 - k0))
            bias_sb = {}
            w_slice(w_tiles["k"], wkT, 0, 1)
            w_slice(w_tiles["q"], wqT, 0, 1)
            # x k-tiles are consumed in index order; gpsimd's stream carries
            # the early tiles, scalar's the late ones, so arrival tracks
            # consumption.  x0 is split across both streams to cut the
            # first-matmul latency.
            nc.gpsimd.dma_start(out=x_sb[0][0:64, :], in_=xT[0:64, :])
            nc.scalar.dma_start(out=x_sb[0][64:P, :], in_=xT[64:P, :])
            for i in range(1, 4):
                nc.gpsimd.dma_start(out=x_sb[i][:], in_=xT[ts(i, P), :])
            for i in range(4, KT):
                nc.scalar.dma_start(out=x_sb[i][:], in_=xT[ts(i, P), :])
            for name, src_ap in (("q", bq), ("k", bk)):
                bias_sb[name] = []
                for mt in range(D_LOC // P):
                    t = const.tile([P, 1], f32, tag=f"b{name}{mt}",
                                   name=f"b{name}{mt}")
                    nc.sync.dma_start(out=t[:], in_=src_ap[ts(mt, P), :])
                    bias_sb[name].append(t)
            w_slice(w_tiles["k"], wkT, 1, 4)
            w_slice(w_tiles["q"], wqT, 1, 4)
            w_slice(w_tiles["k"], wkT, 4, 8)
            w_slice(w_tiles["q"], wqT, 4, 8)
            nc.gpsimd.dma_start(out=w_tiles["v"][:, :, :],
                                in_=wvT[:].rearrange("p (k n) -> p k n", k=KT))
            wp_sb = []
            for i in range(D_LOC // P):
                t = const.tile([P, DIM], dt_mm, tag=f"wp{i}", name=f"wp{i}")
                nc.scalar.dma_start(out=t[:], in_=wpT[ts(i, P), :])
                wp_sb.append(t)

            w_sb = {name: [w_tiles[name][:, i, :] for i in range(KT)]
                    for name in ("k", "q", "v")}

            qk_sb = {}
            for name in ("q", "k"):
                qk_sb[name] = [
                    const.tile([P, N_TOK], dt_mm, tag=f"{name}T{mt}",
                               name=f"{name}T{mt}")
                    for mt in range(D_LOC // P)
                ]
            # v packed per key block, ones-augmented: [:, h, 0:64] = v head h,
            # [:, h, 64:128] = 1.0, so each head's PV matmul also produces the
            # softmax denominator (replicated on psum partitions 64:128) in
            # the otherwise-idle half of the PE output -- no den matmuls.
            vpk_sb = [
                const.tile([P, N_HEADS_LOC, P], dt_mm, tag=f"vp{nt}",
                           name=f"vp{nt}")
                for nt in range(MB)
            ]
            for nt in range(MB):
                nc.vector.memset(vpk_sb[nt][:, :, 64:P], 1.0)
            at_sb = {}

            # ---- emission units; generators double as pipeline fillers ----
            def gen_proj(name, mt, h2, step, halves=(0, 1)):
                """q/k projection group; per-half psum; yields every `step`."""
                n = 0
                for half in halves:
                    ps = ps_pj.tile([P, CH], f32, tag="pj",
                                    name=f"pj_{name}{mt}{h2}{half}")
                    for kt in range(KT):
                        nc.tensor.matmul(
                            ps[:],
                            lhsT=w_sb[name][kt][:, ts(mt, P)],
                            rhs=x_sb[kt][:, ts(2 * h2 + half, CH)],
                            start=(kt == 0), stop=(kt == KT - 1),
                        )
                        n += 1
                        if n % step == 0:
                            yield
                    nc.vector.tensor_scalar_add(
                        qk_sb[name][mt][:, ts(2 * h2 + half, CH)],
                        ps[:], bias_sb[name][mt][:],
                    )
                yield

            def emit_vproj(nt):
                """One v-projection group (one key block), ones kept intact."""
                ps = ps_pj.tile([P, CH], f32, tag="pj", name=f"pj_v{nt}")
                for kt in range(KT):
                    nc.tensor.matmul(
                        ps[:, 0:D_LOC],
                        lhsT=x_sb[kt][:, ts(nt, P)],
                        rhs=w_sb["v"][kt][:],
                        start=(kt == 0), stop=(kt == KT - 1),
                    )
                for h in range(N_HEADS_LOC):
                    nc.vector.tensor_copy(vpk_sb[nt][:, h, 0:64],
                                          ps[:, ts(h, 64)])

            def gen_vproj(nts):
                for nt in nts:
                    emit_vproj(nt)
                    yield

            def gen_outproj(ch):
                """Output projection for chunk ch; one matmul per yield."""
                at_tiles = at_sb[ch]
                for mo in range(DIM // P):
                    pp = ps_pj.tile([P, CH], f32, tag="pj", name=f"pj_o{ch}{mo}")
                    for dt_i in range(2):
                        nc.tensor.matmul(
                            pp[:],
                            lhsT=wp_sb[dt_i][:, ts(mo, P)],
                            rhs=at_tiles[dt_i][:],
                            start=(dt_i == 0), stop=(dt_i == 1),
                        )
                        yield
                    os_sb = work.tile([P, CH], dt_mm, tag="os", bufs=4,
                                      name=f"os{ch}{mo}")
                    nc.vector.tensor_copy(os_sb[:], pp[:])
                    nc.sync.dma_start(out=outT[ts(mo, P), ts(ch, CH)],
                                      in_=os_sb[:])

            def run(gen):
                for _ in gen:
                    pass

            # ---- pre-stream: k00, q00, k01 interleaved in x-arrival (kt)
            # ---- order, then vp0; paced by the input DMAs
            pre_s = {}
            for gname in ("k00", "q00"):
                pre_s[gname] = ps_s.tile([P, 2 * CH], f32, tag="s",
                                         name=f"s_{gname}")
            pre_pj = {}
            for half in range(2):
                pre_pj[half] = ps_pj.tile([P, CH], f32, tag="pj",
                                          name=f"pj_k01{half}")
            for kt in range(KT):
                st, sp = (kt == 0), (kt == KT - 1)
                for gname, wn in (("k00", "k"), ("q00", "q")):
                    for half in range(2):
                        nc.tensor.matmul(
                            pre_s[gname][:, ts(half, CH)],
                            lhsT=w_sb[wn][kt][:, 0:P],
                            rhs=x_sb[kt][:, ts(half, CH)],
                            start=st, stop=sp,
                        )
                for half in range(2):
                    nc.tensor.matmul(
                        pre_pj[half][:],
                        lhsT=w_sb["k"][kt][:, 0:P],
                        rhs=x_sb[kt][:, ts(2 + half, CH)],
                        start=st, stop=sp,
                    )
            for half in range(2):
                nc.vector.tensor_scalar_add(
                    qk_sb["k"][0][:, ts(half, CH)],
                    pre_s["k00"][:, ts(half, CH)], bias_sb["k"][0][:])
                nc.vector.tensor_scalar_add(
                    qk_sb["q"][0][:, ts(half, CH)],
                    pre_s["q00"][:, ts(half, CH)], bias_sb["q"][0][:])
                nc.vector.tensor_scalar_add(
                    qk_sb["k"][0][:, ts(2 + half, CH)],
                    pre_pj[half][:], bias_sb["k"][0][:])

            # ---- flat software-pipelined stream over all key blocks ----
            # QK+exp lead PV by one block globally, so the in-order PE
            # always has score work queued while ACT runs exp, including
            # across (chunk, head-pair) boundaries.
            SEQ = [(0, 0), (1, 0), (0, 1), (1, 1),
                   (2, 0), (2, 1), (3, 0), (3, 1)]
            from itertools import chain

            os3a = []

            def gen_op3_dt0():
                """Chunk-3 outproj, first K-half only (at(3,0) is ready one
                block into position 7); partials staged to SBUF in fp32 so
                the post-stream tail only runs the second K-half."""
                at0 = at_sb[3][0]
                for mo in range(DIM // P):
                    pp = ps_pj.tile([P, CH], f32, tag="pj", name=f"pj_o3a{mo}")
                    nc.tensor.matmul(pp[:], lhsT=wp_sb[0][:, ts(mo, P)],
                                     rhs=at0[:], start=True, stop=True)
                    o = work.tile([P, CH], f32, tag="o3a", bufs=8,
                                  name=f"o3a{mo}")
                    nc.vector.tensor_copy(o[:], pp[:])
                    os3a.append(o)
                    yield

            fillers = {
                0: gen_vproj(range(MB)),
                1: chain(gen_proj("k", 1, 0, 3), gen_proj("k", 1, 1, 3),
                         gen_proj("q", 1, 0, 3)),
                2: chain(gen_proj("q", 0, 1, 2, halves=(0,)),
                         gen_proj("q", 1, 1, 2, halves=(0,))),
                3: None,   # assigned below once at_sb[ch] exists
                4: None,
                5: chain(gen_proj("q", 0, 1, 2, halves=(1,)),
                         gen_proj("q", 1, 1, 2, halves=(1,))),
                6: None,
                7: gen_op3_dt0(),
            }

            blocks = [(i, c, h, mb) for i, (c, h) in enumerate(SEQ)
                      for mb in range(MB)]
            pts = {}
            po_pd = {}
            for g in range(len(blocks) + 1):
                if g < len(blocks):
                    i, c, h, mb = blocks[g]
                    if mb == 0:
                        if i == 3:
                            fillers[3] = gen_outproj(0)
                        elif i == 4:
                            fillers[4] = gen_outproj(1)
                        elif i == 6:
                            fillers[6] = gen_outproj(2)
                        po_pd[(c, h)] = (
                            ps_po.tile([P, CH], f32, tag="poA", name=f"poA{c}{h}"),
                            ps_po.tile([P, CH], f32, tag="poB", name=f"poB{c}{h}"),
                        )
                    f = fillers.get(i)
                    # outproj fillers wait one block for the preceding
                    # pair's normalize to be emitted
                    if f is not None and (i < 3 or mb >= 1):
                        next(f, None)
                    ps = ps_s.tile([P, 1024], f32, tag="s", name=f"s{c}{h}{mb}")
                    nc.tensor.matmul(
                        ps[:, 0:CH],
                        lhsT=qk_sb["k"][h][0:64, ts(mb, P)],
                        rhs=qk_sb["q"][h][0:64, ts(c, CH)],
                    )
                    nc.tensor.matmul(
                        ps[:, CH:1024],
                        lhsT=qk_sb["k"][h][64:P, ts(mb, P)],
                        rhs=qk_sb["q"][h][64:P, ts(c, CH)],
                    )
                    pt = work.tile([P, 1024], dt_mm, tag="pt", bufs=4,
                                   name=f"pt{c}{h}{mb}")
                    nc.scalar.activation(pt[:], ps[:], Exp, scale=SCALE)
                    pts[(c, h, mb)] = pt
                    if mb == MB - 1 and f is not None:
                        run(f)   # drain deferred work before leaving position
                if g >= 1:
                    i2, c2, h2, mb2 = blocks[g - 1]
                    poA, poB = po_pd[(c2, h2)]
                    pt = pts.pop((c2, h2, mb2))
                    st = (mb2 == 0)
                    sp = (mb2 == MB - 1)
                    nc.tensor.matmul(
                        poA[:], lhsT=vpk_sb[mb2][:, 2 * h2, :],
                        rhs=pt[:, 0:CH], start=st, stop=sp,
                    )
                    nc.tensor.matmul(
                        poB[:], lhsT=vpk_sb[mb2][:, 2 * h2 + 1, :],
                        rhs=pt[:, CH:1024], start=st, stop=sp,
                    )
                    if sp:
                        # normalize.  HW constraints (micro-tested): two-input
                        # DVE ops need equal input base partitions (out may
                        # shift); reciprocal_approx_fast needs base 0.  So:
                        # stage po to SBUF (also frees the psum slot for the
                        # next pair), cross-copy the den replicas to base 0,
                        # one recip, two aligned muls.  The last pair skips
                        # the staging (no successor needs its psum slots) to
                        # shorten the end-of-stream critical path.
                        del po_pd[(c2, h2)]
                        last = (c2, h2) == SEQ[-1]
                        den = work.tile([64, 2 * CH], f32, tag="den", bufs=2,
                                        name=f"den{c2}{h2}")
                        if last:
                            pvA, pvB = poA[0:64, :], poB[0:64, :]
                            nc.vector.tensor_copy(den[:, 0:CH], poA[64:P, :])
                            nc.vector.tensor_copy(den[:, CH:2 * CH],
                                                  poB[64:P, :])
                        else:
                            poS = work.tile([P, 2 * CH], f32, tag="poS",
                                            bufs=2, name=f"poS{c2}{h2}")
                            nc.vector.tensor_copy(poS[:, 0:CH], poA[:])
                            nc.vector.tensor_copy(poS[:, CH:2 * CH], poB[:])
                            pvA, pvB = poS[0:64, 0:CH], poS[0:64, CH:2 * CH]
                            nc.vector.tensor_copy(den[:], poS[64:P, :])
                        rec = work.tile([64, 2 * CH], f32, tag="bc", bufs=2,
                                        name=f"rec{c2}{h2}")
                        nc.vector.reciprocal_approx_fast(rec[:], den[:])
                        at = work.tile([P, CH], dt_mm, tag="at", bufs=4,
                                       name=f"at{c2}{h2}")
                        nc.vector.tensor_mul(at[0:64, :], pvA, rec[:, 0:CH])
                        nc.vector.tensor_mul(at[64:P, :], pvB,
                                             rec[:, CH:2 * CH])
                        at_sb.setdefault(c2, []).append(at)
            # post-stream tail: only the second K-half of chunk 3's outproj
            at1 = at_sb[3][1]
            for mo in range(DIM // P):
                pp = ps_pj.tile([P, CH], f32, tag="pj", name=f"pj_o3b{mo}")
                nc.tensor.matmul(pp[:], lhsT=wp_sb[1][:, ts(mo, P)],
                                 rhs=at1[:], start=True, stop=True)
                os_sb = work.tile([P, CH], dt_mm, tag="os", bufs=4,
                                  name=f"os3{mo}")
                nc.vector.tensor_add(os_sb[:], os3a[mo][:], pp[:])
                nc.sync.dma_start(out=outT[ts(mo, P), ts(3, CH)],
                                  in_=os_sb[:])

    nc.compile()
    return nc


def _get_nc():
    if "nc" not in _NC_CACHE:
        _NC_CACHE["nc"] = build_nc(DT_MM_NAME)
    return _NC_CACHE["nc"]


def make_in_maps(x, Wq, bq, Wk, bk, Wv, bv, Wp, bp, dt_mm_name="bfloat16"):
    """Shard full inputs into 8 per-core input maps."""
    f = np.float32
    if dt_mm_name == "bfloat16":
        import ml_dtypes
        mmt = ml_dtypes.bfloat16
    else:
        mmt = np.float32
    x = np.asarray(x, f)
    xT = [np.ascontiguousarray(x[b].T).astype(mmt) for b in range(x.shape[0])]
    WqT = np.asarray(Wq, f).T
    WkT = np.asarray(Wk, f).T
    WvT = np.asarray(Wv, f).T
    WpT = np.asarray(Wp, f).T
    def pretile(w):
        # [1024, 256] -> [128, 8*256]: partition p holds all 8 k-tiles
        # contiguously so DMA descriptors are 4KB DRAM runs
        return np.ascontiguousarray(
            w.reshape(KT, P, D_LOC).transpose(1, 0, 2).reshape(P, KT * D_LOC)
        ).astype(mmt)

    in_maps = []
    for c in range(N_CORES):
        b, r = divmod(c, 4)
        sl = slice(D_LOC * r, D_LOC * (r + 1))
        in_maps.append({
            "xT": xT[b],
            "wqT": pretile(WqT[:, sl]),
            "wkT": pretile(WkT[:, sl]),
            "wvT": pretile(WvT[:, sl]),
            "wpT": np.ascontiguousarray(WpT[sl, :]).astype(mmt),
            "bq": np.asarray(bq, f)[sl].reshape(D_LOC, 1).copy(),
            "bk": np.asarray(bk, f)[sl].reshape(D_LOC, 1).copy(),
        })
    return in_maps


def assemble_output(results, Wv, bv, Wp, bp):
    """Sum TP partials, transpose back, add folded biases."""
    f = np.float32
    bp_eff = np.asarray(bv, f) @ np.asarray(Wp, f).T + np.asarray(bp, f)
    out = np.empty((2, N_TOK, DIM), f)
    for b in range(2):
        acc = results[4 * b]["outT"].astype(f)
        for r in range(1, 4):
            acc = acc + results[4 * b + r]["outT"].astype(f)
        out[b] = acc.T + bp_eff
    return out


DT_MM_NAME = "bfloat16"


def kernel(x, Wq, bq, Wk, bk, Wv, bv, Wp, bp):
    from concourse.bass_utils import run_bass_kernel_spmd
    nc = _get_nc()
    in_maps = make_in_maps(x, Wq, bq, Wk, bk, Wv, bv, Wp, bp, DT_MM_NAME)
    res = run_bass_kernel_spmd(nc, in_maps, list(range(N_CORES)))
    return assemble_output(res.results, Wv, bv, Wp, bp)


# revision 20
# speedup vs baseline: 1.2046x; 1.2046x over previous
"""Fused multi-head attention on 8 TRN2 NeuronCores.

Problem: x[2,2048,1024] -> q,k,v = x@W.T+b (16 heads x 64), softmax(q k^T/8) v,
then out @ Wp.T + bp.

Sharding: data-parallel over batch (2) x tensor-parallel over heads (4 ranks x
4 heads = 256 dims, Megatron-style).  Core c handles batch c//4, head-rank c%4.
The proj partial sums are reduced on the host (numpy), and the v-bias and
proj-bias are folded into one host-side vector bp_eff = bv @ Wp.T + bp.

Per-core layouts (host pre-transposes/pre-tiles, all DMA rows are >=512B
contiguous DRAM runs):
  xT  [1024, 2048]  x[b].T
  wqT/wkT/wvT [128, 8*256]  W.T slice pre-tiled so partition p holds all 8
                            contraction tiles contiguously
  wpT [256, 1024]           Wp.T rows for this rank's 256 dims
  bq/bk [256, 1]
  outT [1024, 2048] bf16 partial (x[b] @ ..).T, missing bv/bp contributions

Kernel math per core (all matmul operands bfloat16, fp32 PSUM accumulate):
  qT = wqT.T @ xT + bq   [256, 2048]  (transposed layout, d on partitions)
  kT = wkT.T @ xT + bk   [256, 2048]
  v  = xT.T @ wvT        [2048, 256]  packed per key-block with a block of
                         ones appended: vpk[nt] = [128 keys, 4 heads, 128]
                         where cols 0:64 = v, cols 64:128 = 1.0
  attention runs as ONE flat software-pipelined stream over all 128
  (n-chunk, head-pair, key-block) blocks; per block:
     sT[m, n] = kT.T @ qT    two heads row-packed (K=64 matmuls)
     p = exp(sT / 8)         ACT, one [128,1024] instr, both heads
     po_h[:, n] += vpk[mb,h].T @ p_h   per head: partitions 0:64 = P@V,
                              partitions 64:128 = softmax denominator
                              (the ones-columns), i.e. the denominator
                              rides along in the otherwise-idle half of
                              the PE output -- no separate den matmuls.
  PV trails its block's exp by one position globally, so the in-order
  PE always has the next score matmuls queued while ACT runs exp.  The
  q/k/v projections and the output projection are emitted as "filler"
  generators pumped one slice per block inside the stream, absorbing the
  PE idle gaps of the ACT-paced phase.
  attnT = po[0:64] * reciprocal_approx_fast(po[64:128]) per head
  outT += wpT.T @ attnT    [1024, n-chunk], per 128-row tile, bf16 out
"""

import numpy as np

DIM = 1024
N_TOK = 2048
N_HEADS_LOC = 4       # heads per core
D_LOC = 256           # local q/k/v dims per core
SCALE = 64 ** -0.5
P = 128
CH = 512              # n-chunk (moving free dim)
NCH = N_TOK // CH     # 4
KT = DIM // P         # 8 contraction tiles for qkv/proj
MB = N_TOK // P       # 16 key blocks
N_CORES = 8

_NC_CACHE = {}


def build_nc(dt_mm_name="bfloat16"):
    import concourse.mybir as mybir
    import concourse.tile as tile
    from concourse import bacc
    from concourse.bass import ts

    f32 = mybir.dt.float32
    dt_mm = getattr(mybir.dt, dt_mm_name)
    Exp = mybir.ActivationFunctionType.Exp

    nc = bacc.Bacc("TRN2", target_bir_lowering=False, debug=False,
                   num_devices=N_CORES)
    xT = nc.dram_tensor("xT", [DIM, N_TOK], dt_mm, kind="ExternalInput").ap()
    wqT = nc.dram_tensor("wqT", [P, KT * D_LOC], dt_mm, kind="ExternalInput").ap()
    wkT = nc.dram_tensor("wkT", [P, KT * D_LOC], dt_mm, kind="ExternalInput").ap()
    wvT = nc.dram_tensor("wvT", [P, KT * D_LOC], dt_mm, kind="ExternalInput").ap()
    wpT = nc.dram_tensor("wpT", [D_LOC, DIM], dt_mm, kind="ExternalInput").ap()
    bq = nc.dram_tensor("bq", [D_LOC, 1], f32, kind="ExternalInput").ap()
    bk = nc.dram_tensor("bk", [D_LOC, 1], f32, kind="ExternalInput").ap()
    outT = nc.dram_tensor("outT", [DIM, N_TOK], dt_mm, kind="ExternalOutput").ap()

    with tile.TileContext(nc) as tc:
        with (
            tc.tile_pool(name="const", bufs=1) as const,
            tc.tile_pool(name="work", bufs=2) as work,
            # scores + pre-stream k00/q00 accumulators: 2x [128,1024] = 4 banks
            tc.tile_pool(name="ps_s", bufs=2, space="PSUM") as ps_s,
            # filler ring (proj halves, vproj, outproj): 2x [128,512] = 2 banks
            tc.tile_pool(name="ps_pj", bufs=2, space="PSUM") as ps_pj,
            # PV+den accumulators, one per head of the active pair: 2 banks
            tc.tile_pool(name="ps_po", bufs=1, space="PSUM") as ps_po,
        ):
            # ---- persistent SBUF state ----
            w_tiles = {}
            for name in ("k", "q", "v"):
                w_tiles[name] = const.tile([P, KT, D_LOC], dt_mm, tag=f"w{name}",
                                           name=f"w{name}")
            x_sb = [const.tile([P, N_TOK], dt_mm, tag=f"x{i}", name=f"x{i}")
                    for i in range(KT)]

            # DMA issue order: each engine's dma_starts serialize (~0.65us
            # each) and the 16 queues round-robin all in-flight transfers, so
            # total wire time (~6MB at ~320GB/s aggregate = 19us) bounds the
            # head.  Only x/wk/wq (+biases) gate the pre-stream, so they go
            # first; wv is needed one block into the stream and wp only at
            # position 3, so they queue behind everything else.
            def w_slice(dst, src_ap, k0, k1):
                nc.sync.dma_start(
                    out=dst[:, k0:k1, :],
                    in_=src_ap[:, k0 * D_LOC:k1 * D_LOC].rearrange(
                        "p (k n) -> p k n", k=k1 - k0))
            bias_sb = {}
            w_slice(w_tiles["k"], wkT, 0, 1)
            w_slice(w_tiles["q"], wqT, 0, 1)
            # x k-tiles are consumed in index order; gpsimd's stream carries
            # the early tiles, scalar's the late ones, so arrival tracks
            # consumption.  x0 is split across both streams to cut the
            # first-matmul latency.
            nc.gpsimd.dma_start(out=x_sb[0][0:64, :], in_=xT[0:64, :])
            nc.scalar.dma_start(out=x_sb[0][64:P, :], in_=xT[64:P, :])
            for i in range(1, 4):
                nc.gpsimd.dma_start(out=x_sb[i][:], in_=xT[ts(i, P), :])
            for i in range(4, KT):
                nc.scalar.dma_start(out=x_sb[i][:], in_=xT[ts(i, P), :])
            for name, src_ap in (("q", bq), ("k", bk)):
                bias_sb[name] = []
                for mt in range(D_LOC // P):
                    t = const.tile([P, 1], f32, tag=f"b{name}{mt}",
                                   name=f"b{name}{mt}")
                    nc.sync.dma_start(out=t[:], in_=src_ap[ts(mt, P), :])
                    bias_sb[name].append(t)
            # per-k-tile alternating wk/wq so the pre-stream (which consumes
            # k-tiles in order) never waits on a large trailing weight dma
            for kt in range(1, KT):
                w_slice(w_tiles["k"], wkT, kt, kt + 1)
                w_slice(w_tiles["q"], wqT, kt, kt + 1)
            nc.gpsimd.dma_start(out=w_tiles["v"][:, :, :],
                                in_=wvT[:].rearrange("p (k n) -> p k n", k=KT))
            wp_sb = []
            for i in range(D_LOC // P):
                t = const.tile([P, DIM], dt_mm, tag=f"wp{i}", name=f"wp{i}")
                nc.scalar.dma_start(out=t[:], in_=wpT[ts(i, P), :])
                wp_sb.append(t)

            w_sb = {name: [w_tiles[name][:, i, :] for i in range(KT)]
                    for name in ("k", "q", "v")}

            qk_sb = {}
            for name in ("q", "k"):
                qk_sb[name] = [
                    const.tile([P, N_TOK], dt_mm, tag=f"{name}T{mt}",
                               name=f"{name}T{mt}")
                    for mt in range(D_LOC // P)
                ]
            # v packed per key block, ones-augmented: [:, h, 0:64] = v head h,
            # [:, h, 64:128] = 1.0, so each head's PV matmul also produces the
            # softmax denominator (replicated on psum partitions 64:128) in
            # the otherwise-idle half of the PE output -- no den matmuls.
            vpk_sb = [
                const.tile([P, N_HEADS_LOC, P], dt_mm, tag=f"vp{nt}",
                           name=f"vp{nt}")
                for nt in range(MB)
            ]
            for nt in range(MB):
                nc.vector.memset(vpk_sb[nt][:, :, 64:P], 1.0)
            at_sb = {}

            # ---- emission units; generators double as pipeline fillers ----
            def gen_proj(name, mt, h2, step, halves=(0, 1)):
                """q/k projection group; per-half psum; yields every `step`."""
                n = 0
                for half in halves:
                    ps = ps_pj.tile([P, CH], f32, tag="pj",
                                    name=f"pj_{name}{mt}{h2}{half}")
                    for kt in range(KT):
                        nc.tensor.matmul(
                            ps[:],
                            lhsT=w_sb[name][kt][:, ts(mt, P)],
                            rhs=x_sb[kt][:, ts(2 * h2 + half, CH)],
                            start=(kt == 0), stop=(kt == KT - 1),
                        )
                        n += 1
                        if n % step == 0:
                            yield
                    nc.vector.tensor_scalar_add(
                        qk_sb[name][mt][:, ts(2 * h2 + half, CH)],
                        ps[:], bias_sb[name][mt][:],
                    )
                yield

            def emit_vproj(nt):
                """One v-projection group (one key block), ones kept intact."""
                ps = ps_pj.tile([P, CH], f32, tag="pj", name=f"pj_v{nt}")
                for kt in range(KT):
                    nc.tensor.matmul(
                        ps[:, 0:D_LOC],
                        lhsT=x_sb[kt][:, ts(nt, P)],
                        rhs=w_sb["v"][kt][:],
                        start=(kt == 0), stop=(kt == KT - 1),
                    )
                for h in range(N_HEADS_LOC):
                    nc.vector.tensor_copy(vpk_sb[nt][:, h, 0:64],
                                          ps[:, ts(h, 64)])

            def gen_vproj(nts):
                for nt in nts:
                    emit_vproj(nt)
                    yield

            def gen_outproj(ch):
                """Output projection for chunk ch; one matmul per yield."""
                at_tiles = at_sb[ch]
                for mo in range(DIM // P):
                    pp = ps_pj.tile([P, CH], f32, tag="pj", name=f"pj_o{ch}{mo}")
                    for dt_i in range(2):
                        nc.tensor.matmul(
                            pp[:],
                            lhsT=wp_sb[dt_i][:, ts(mo, P)],
                            rhs=at_tiles[dt_i][:],
                            start=(dt_i == 0), stop=(dt_i == 1),
                        )
                        yield
                    os_sb = work.tile([P, CH], dt_mm, tag="os", bufs=4,
                                      name=f"os{ch}{mo}")
                    nc.vector.tensor_copy(os_sb[:], pp[:])
                    nc.sync.dma_start(out=outT[ts(mo, P), ts(ch, CH)],
                                      in_=os_sb[:])

            def run(gen):
                for _ in gen:
                    pass

            # ---- pre-stream: k00, q00, k01 interleaved in x-arrival (kt)
            # ---- order, then vp0; paced by the input DMAs
            pre_s = {}
            for gname in ("k00", "q00"):
                pre_s[gname] = ps_s.tile([P, 2 * CH], f32, tag="s",
                                         name=f"s_{gname}")
            pre_pj = {}
            for half in range(2):
                pre_pj[half] = ps_pj.tile([P, CH], f32, tag="pj",
                                          name=f"pj_k01{half}")
            for kt in range(KT):
                st, sp = (kt == 0), (kt == KT - 1)
                # k-work first: wk[kt] lands before wq[kt] in the dma order
                for half in range(2):
                    nc.tensor.matmul(
                        pre_s["k00"][:, ts(half, CH)],
                        lhsT=w_sb["k"][kt][:, 0:P],
                        rhs=x_sb[kt][:, ts(half, CH)],
                        start=st, stop=sp,
                    )
                for half in range(2):
                    nc.tensor.matmul(
                        pre_pj[half][:],
                        lhsT=w_sb["k"][kt][:, 0:P],
                        rhs=x_sb[kt][:, ts(2 + half, CH)],
                        start=st, stop=sp,
                    )
                for half in range(2):
                    nc.tensor.matmul(
                        pre_s["q00"][:, ts(half, CH)],
                        lhsT=w_sb["q"][kt][:, 0:P],
                        rhs=x_sb[kt][:, ts(half, CH)],
                        start=st, stop=sp,
                    )
            for half in range(2):
                nc.vector.tensor_scalar_add(
                    qk_sb["k"][0][:, ts(half, CH)],
                    pre_s["k00"][:, ts(half, CH)], bias_sb["k"][0][:])
                nc.vector.tensor_scalar_add(
                    qk_sb["q"][0][:, ts(half, CH)],
                    pre_s["q00"][:, ts(half, CH)], bias_sb["q"][0][:])
                nc.vector.tensor_scalar_add(
                    qk_sb["k"][0][:, ts(2 + half, CH)],
                    pre_pj[half][:], bias_sb["k"][0][:])

            # ---- flat software-pipelined stream over all key blocks ----
            # QK+exp lead PV by one block globally, so the in-order PE
            # always has score work queued while ACT runs exp, including
            # across (chunk, head-pair) boundaries.
            SEQ = [(0, 0), (1, 0), (0, 1), (1, 1),
                   (2, 0), (2, 1), (3, 0), (3, 1)]
            from itertools import chain

            os3a = []

            def gen_op3_dt0():
                """Chunk-3 outproj, first K-half only (at(3,0) is ready one
                block into position 7); partials staged to SBUF in fp32 so
                the post-stream tail only runs the second K-half."""
                at0 = at_sb[3][0]
                for mo in range(DIM // P):
                    pp = ps_pj.tile([P, CH], f32, tag="pj", name=f"pj_o3a{mo}")
                    nc.tensor.matmul(pp[:], lhsT=wp_sb[0][:, ts(mo, P)],
                                     rhs=at0[:], start=True, stop=True)
                    o = work.tile([P, CH], f32, tag="o3a", bufs=8,
                                  name=f"o3a{mo}")
                    nc.vector.tensor_copy(o[:], pp[:])
                    os3a.append(o)
                    yield

            fillers = {
                0: gen_vproj(range(MB)),
                1: chain(gen_proj("k", 1, 0, 3), gen_proj("k", 1, 1, 3),
                         gen_proj("q", 1, 0, 3)),
                2: chain(gen_proj("q", 0, 1, 2, halves=(0,)),
                         gen_proj("q", 1, 1, 2, halves=(0,))),
                3: None,   # assigned below once at_sb[ch] exists
                4: None,
                5: chain(gen_proj("q", 0, 1, 2, halves=(1,)),
                         gen_proj("q", 1, 1, 2, halves=(1,))),
                6: None,
                7: gen_op3_dt0(),
            }

            blocks = [(i, c, h, mb) for i, (c, h) in enumerate(SEQ)
                      for mb in range(MB)]
            pts = {}
            po_pd = {}
            for g in range(len(blocks) + 1):
                if g < len(blocks):
                    i, c, h, mb = blocks[g]
                    if mb == 0:
                        if i == 3:
                            fillers[3] = gen_outproj(0)
                        elif i == 4:
                            fillers[4] = gen_outproj(1)
                        elif i == 6:
                            fillers[6] = gen_outproj(2)
                        po_pd[(c, h)] = (
                            ps_po.tile([P, CH], f32, tag="poA", name=f"poA{c}{h}"),
                            ps_po.tile([P, CH], f32, tag="poB", name=f"poB{c}{h}"),
                        )
                    f = fillers.get(i)
                    # outproj fillers wait one block for the preceding
                    # pair's normalize to be emitted
                    if f is not None and (i < 3 or mb >= 1):
                        next(f, None)
                    ps = ps_s.tile([P, 1024], f32, tag="s", name=f"s{c}{h}{mb}")
                    nc.tensor.matmul(
                        ps[:, 0:CH],
                        lhsT=qk_sb["k"][h][0:64, ts(mb, P)],
                        rhs=qk_sb["q"][h][0:64, ts(c, CH)],
                    )
                    nc.tensor.matmul(
                        ps[:, CH:1024],
                        lhsT=qk_sb["k"][h][64:P, ts(mb, P)],
                        rhs=qk_sb["q"][h][64:P, ts(c, CH)],
                    )
                    pt = work.tile([P, 1024], dt_mm, tag="pt", bufs=4,
                                   name=f"pt{c}{h}{mb}")
                    nc.scalar.activation(pt[:], ps[:], Exp, scale=SCALE)
                    pts[(c, h, mb)] = pt
                    if mb == MB - 1 and f is not None:
                        run(f)   # drain deferred work before leaving position
                if g >= 1:
                    i2, c2, h2, mb2 = blocks[g - 1]
                    poA, poB = po_pd[(c2, h2)]
                    pt = pts.pop((c2, h2, mb2))
                    st = (mb2 == 0)
                    sp = (mb2 == MB - 1)
                    nc.tensor.matmul(
                        poA[:], lhsT=vpk_sb[mb2][:, 2 * h2, :],
                        rhs=pt[:, 0:CH], start=st, stop=sp,
                    )
                    nc.tensor.matmul(
                        poB[:], lhsT=vpk_sb[mb2][:, 2 * h2 + 1, :],
                        rhs=pt[:, CH:1024], start=st, stop=sp,
                    )
                    if sp:
                        # normalize.  HW constraints (micro-tested): two-input
                        # DVE ops need equal input base partitions (out may
                        # shift); reciprocal_approx_fast needs base 0.  So:
                        # stage po to SBUF (also frees the psum slot for the
                        # next pair), cross-copy the den replicas to base 0,
                        # one recip, two aligned muls.  The last pair skips
                        # the staging (no successor needs its psum slots) to
                        # shorten the end-of-stream critical path.
                        del po_pd[(c2, h2)]
                        last = (c2, h2) == SEQ[-1]
                        den = work.tile([64, 2 * CH], f32, tag="den", bufs=2,
                                        name=f"den{c2}{h2}")
                        if last:
                            pvA, pvB = poA[0:64, :], poB[0:64, :]
                            nc.vector.tensor_copy(den[:, 0:CH], poA[64:P, :])
                            nc.vector.tensor_copy(den[:, CH:2 * CH],
                                                  poB[64:P, :])
                        else:
                            poS = work.tile([P, 2 * CH], f32, tag="poS",
                                            bufs=2, name=f"poS{c2}{h2}")
                            nc.vector.tensor_copy(poS[:, 0:CH], poA[:])
                            nc.vector.tensor_copy(poS[:, CH:2 * CH], poB[:])
                            pvA, pvB = poS[0:64, 0:CH], poS[0:64, CH:2 * CH]
                            nc.vector.tensor_copy(den[:], poS[64:P, :])
                        rec = work.tile([64, 2 * CH], f32, tag="bc", bufs=2,
                                        name=f"rec{c2}{h2}")
                        nc.vector.reciprocal_approx_fast(rec[:], den[:])
                        at = work.tile([P, CH], dt_mm, tag="at", bufs=4,
                                       name=f"at{c2}{h2}")
                        nc.vector.tensor_mul(at[0:64, :], pvA, rec[:, 0:CH])
                        nc.vector.tensor_mul(at[64:P, :], pvB,
                                             rec[:, CH:2 * CH])
                        at_sb.setdefault(c2, []).append(at)
            # post-stream tail: only the second K-half of chunk 3's outproj
            at1 = at_sb[3][1]
            for mo in range(DIM // P):
                pp = ps_pj.tile([P, CH], f32, tag="pj", name=f"pj_o3b{mo}")
                nc.tensor.matmul(pp[:], lhsT=wp_sb[1][:, ts(mo, P)],
                                 rhs=at1[:], start=True, stop=True)
                os_sb = work.tile([P, CH], dt_mm, tag="os", bufs=4,
                                  name=f"os3{mo}")
                nc.vector.tensor_add(os_sb[:], os3a[mo][:], pp[:])
                nc.sync.dma_start(out=outT[ts(mo, P), ts(3, CH)],
                                  in_=os_sb[:])

    nc.compile()
    return nc


def _get_nc():
    if "nc" not in _NC_CACHE:
        _NC_CACHE["nc"] = build_nc(DT_MM_NAME)
    return _NC_CACHE["nc"]


def make_in_maps(x, Wq, bq, Wk, bk, Wv, bv, Wp, bp, dt_mm_name="bfloat16"):
    """Shard full inputs into 8 per-core input maps."""
    f = np.float32
    if dt_mm_name == "bfloat16":
        import ml_dtypes
        mmt = ml_dtypes.bfloat16
    else:
        mmt = np.float32
    x = np.asarray(x, f)
    xT = [np.ascontiguousarray(x[b].T).astype(mmt) for b in range(x.shape[0])]
    WqT = np.asarray(Wq, f).T
    WkT = np.asarray(Wk, f).T
    WvT = np.asarray(Wv, f).T
    WpT = np.asarray(Wp, f).T
    def pretile(w):
        # [1024, 256] -> [128, 8*256]: partition p holds all 8 k-tiles
        # contiguously so DMA descriptors are 4KB DRAM runs
        return np.ascontiguousarray(
            w.reshape(KT, P, D_LOC).transpose(1, 0, 2).reshape(P, KT * D_LOC)
        ).astype(mmt)

    in_maps = []
    for c in range(N_CORES):
        b, r = divmod(c, 4)
        sl = slice(D_LOC * r, D_LOC * (r + 1))
        in_maps.append({
            "xT": xT[b],
            "wqT": pretile(WqT[:, sl]),
            "wkT": pretile(WkT[:, sl]),
            "wvT": pretile(WvT[:, sl]),
            "wpT": np.ascontiguousarray(WpT[sl, :]).astype(mmt),
            "bq": np.asarray(bq, f)[sl].reshape(D_LOC, 1).copy(),
            "bk": np.asarray(bk, f)[sl].reshape(D_LOC, 1).copy(),
        })
    return in_maps


def assemble_output(results, Wv, bv, Wp, bp):
    """Sum TP partials, transpose back, add folded biases."""
    f = np.float32
    bp_eff = np.asarray(bv, f) @ np.asarray(Wp, f).T + np.asarray(bp, f)
    out = np.empty((2, N_TOK, DIM), f)
    for b in range(2):
        acc = results[4 * b]["outT"].astype(f)
        for r in range(1, 4):
            acc = acc + results[4 * b + r]["outT"].astype(f)
        out[b] = acc.T + bp_eff
    return out


DT_MM_NAME = "bfloat16"


def kernel(x, Wq, bq, Wk, bk, Wv, bv, Wp, bp):
    from concourse.bass_utils import run_bass_kernel_spmd
    nc = _get_nc()
    in_maps = make_in_maps(x, Wq, bq, Wk, bk, Wv, bv, Wp, bp, DT_MM_NAME)
    res = run_bass_kernel_spmd(nc, in_maps, list(range(N_CORES)))
    return assemble_output(res.results, Wv, bv, Wp, bp)
